# revision 6
# baseline (speedup 1.0000x reference)
"""Chamfer distance kernel for Trainium2 (8 NeuronCores).

Strategy
--------
dist[b,i,j] = ||pred[b,j] - gt[b,i]||.  Mins are taken over *negated
squared* distances (so reductions are max); sqrt/means happen on host.

neg_sq is produced in PSUM by one augmented K=24 bf16 matmul per
[128 x 512] block (fp32 operands split into bf16 triples; 4 blocks of a
[128 x 2048] strip run in distinct 32-row PE groups).

Sharding: gt rows split across 8 cores (1024 rows/core/batch). Per core
64 strips ([2 batches x 8 row-tiles] x [4 col blocks]).  Each strip must
leave PSUM through an element-paced engine, so the work is balanced
across all three:
  - ScalarE (ACT) evicts 44 strips (Copy fp32->fp16, ~2.0us each),
  - DVE evicts 20 strips via tensor_scalar+accum (rowmax falls out free),
  - row-tile quads are folded into a per-batch running colmax [128,8192]
    with scalar_tensor_tensor: fp16 all-SBUF => DVE 4x mode (~2.2us per
    8192-wide quad); 7 of 14 folds go to the otherwise idle GpSimd,
  - rowmax for ACT-evicted strips: one 4x tensor_scalar+accum pass over
    the contiguous slices of the quad.
Chain-initial tiles (t=0) evict straight into the running buffer (no
fold, no memset).  Final tiles (t=7) fold per-2048-slice so the output
DMA overlaps the last folds.

Outputs per core: rowmax accum [128, 64] fp32 and colmax [128, B*8192]
fp16.  The host folds partitions/cores, applies sqrt and means (f64).
"""

import os
import sys
import numpy as np
import ml_dtypes

# ---------------------------------------------------------------------------
# problem constants (hardcoded per spec: pred/gt [2, 8192, 3] fp32)
B = 2
N = 8192
NCORES = 8
GPC = N // NCORES          # gt rows per core per batch = 1024
RT = GPC // 128            # row tiles per batch per core = 8
CB = 4                     # col blocks per batch (each 2048 preds)
CBW = N // CB              # col block width = 2048
K = 24                     # contraction rows of the augmented matmul

_BF16 = ml_dtypes.bfloat16


def _ensure_concourse():
    for p in ("/root/.axon_site", "/root/.axon_site/_ro/trn_rl_repo",
              "/root/.axon_site/_ro/pypackages", "/opt/trn_rl_repo"):
        if os.path.isdir(p) and p not in sys.path:
            sys.path.append(p)


def _split3(x64):
    """Split a float64 array into three bf16 components summing to ~24 bits."""
    h = x64.astype(_BF16)
    r = x64 - h.astype(np.float64)
    m = r.astype(_BF16)
    r2 = r - m.astype(np.float64)
    l = r2.astype(_BF16)
    return h, m, l


def _build_aug(pred, gt):
    """Build aug_pred [K, B*N] and aug_gt [K, B*N] bf16 host arrays.

    Row pairing k: lhsT[k] (gt side) x rhs[k] (pred side):
      0-2   gh . Ph      3-5   gh . Pm      6-8   gm . Ph
      9-11  gh . Pl     12-14  gl . Ph     15-17  gm . Pm
      18-20 gsq{h,m,l} . (-1)              21-23  1 . (-psq{h,m,l})
    where P = 2*pred.
    """
    g64 = gt.astype(np.float64).reshape(B * N, 3)
    P64 = (2.0 * pred.astype(np.float64)).reshape(B * N, 3)
    gsq = (gt.astype(np.float32) ** 2).sum(-1, dtype=np.float32).astype(np.float64).reshape(B * N)
    psq = (pred.astype(np.float32) ** 2).sum(-1, dtype=np.float32).astype(np.float64).reshape(B * N)

    gh, gm, gl = _split3(g64)
    Ph, Pm, Pl = _split3(P64)
    gsqh, gsqm, gsql = _split3(gsq)
    psqh, psqm, psql = _split3(psq)

    one = np.ones(B * N, _BF16)
    neg1 = np.full(B * N, -1.0, _BF16)

    def rows3(a):  # [B*N, 3] -> 3 rows
        return [a[:, 0], a[:, 1], a[:, 2]]

    aug_gt = np.stack(
        rows3(gh) + rows3(gh) + rows3(gm) + rows3(gh) + rows3(gl) + rows3(gm)
        + [gsqh, gsqm, gsql, one, one, one], axis=0)
    aug_pred = np.stack(
        rows3(Ph) + rows3(Pm) + rows3(Ph) + rows3(Pl) + rows3(Ph) + rows3(Pm)
        + [neg1, neg1, neg1, -psqh, -psqm, -psql], axis=0)
    assert aug_gt.shape == (K, B * N) and aug_pred.shape == (K, B * N)
    return aug_gt, aug_pred


# per-tile strip classes: which cb indices the DVE evicts (rest go to ACT),
# and the contiguous ACT span [lo, hi) used for the quad rowmax pass.
def _tile_plan(t):
    return (3,), (0, 3)        # 3 ACT + 1 DVE, ACT span cb 0..2


def build_nc():
    """Trace + compile the single-program SPMD kernel. Returns the Bacc."""
    _ensure_concourse()
    from contextlib import ExitStack
    import concourse.tile as tile
    from concourse import bacc, mybir

    f32 = mybir.dt.float32
    bf16 = mybir.dt.bfloat16
    f16 = mybir.dt.float16
    MAX = mybir.AluOpType.max
    ADD = mybir.AluOpType.add

    nc = bacc.Bacc("TRN2", target_bir_lowering=False, debug=False,
                   enable_asserts=False, num_devices=NCORES)
    ag_d = nc.dram_tensor("aug_gt", [K, B * GPC], bf16, kind="ExternalInput").ap()
    ap_d = nc.dram_tensor("aug_pred", [K, B * N], bf16, kind="ExternalInput").ap()
    # rowmax accum columns: col = (b*RT + t)*4 + slot (slot: ACT pass or cb)
    rmax_d = nc.dram_tensor("rowmax_out", [128, B * RT * 4], f32,
                            kind="ExternalOutput").ap()
    # colmax partials folded over all row tiles; host folds partitions+cores.
    cmax_d = nc.dram_tensor("colmax_out", [128, B * N], f16,
                            kind="ExternalOutput").ap()

    with tile.TileContext(nc) as tc, ExitStack() as ctx:
        const_pool = ctx.enter_context(tc.tile_pool(name="const", bufs=1))
        psum_pool = ctx.enter_context(tc.tile_pool(name="ps", bufs=2, space="PSUM"))
        qpool = ctx.enter_context(tc.tile_pool(name="quad", bufs=3))
        rpool = ctx.enter_context(tc.tile_pool(name="run", bufs=1))

        # operands replicated at partition bases 0/32/64/96 so each strip's 4
        # matmuls occupy distinct 32-row PE row groups and run concurrently.
        ag = const_pool.tile([96 + K, B * GPC], bf16)
        apt = const_pool.tile([96 + K, B * N], bf16)
        for rg in range(4):
            nc.sync.dma_start(ag[32 * rg:32 * rg + K, :], ag_d[:])
        for b in range(B):
            for cb in range(CB):
                ccol = b * N + cb * CBW
                for rg in range(4):
                    nc.sync.dma_start(apt[32 * rg:32 * rg + K, ccol:ccol + CBW],
                                      ap_d[:, ccol:ccol + CBW])

        rfin = const_pool.tile([128, B * RT * 4], f32)
        nc.vector.memset(rfin[:], -3.0e38)
        running = [rpool.tile([128, N], f16, tag=f"run{b}", name=f"run{b}")
                   for b in range(B)]
        dummy = const_pool.tile([128, 3 * CBW], f16)

        # global tile order: batches interleaved so both running chains
        # advance together; b1 finishes one tile before b0.
        order = []
        for t in range(RT):
            for b in range(B):
                order.append((b, t) if t % 2 == 0 else (1 - b, t))

        fold_ctr = 0
        for (b, t) in order:
            wcol = (b * RT + t) * 128
            rbase = (b * RT + t) * 4
            dve_cbs, (alo, ahi) = _tile_plan(t)
            init = (t == 0)
            quad = running[b] if init else qpool.tile([128, N], f16, tag="q")
            for cb in range(CB):
                ccol = b * N + cb * CBW
                psum = psum_pool.tile([128, CBW], f32, tag="ps")
                for n in range(4):
                    nc.tensor.matmul(
                        psum[:, n * 512:(n + 1) * 512],
                        lhsT=ag[32 * n:32 * n + K, wcol:wcol + 128],
                        rhs=apt[32 * n:32 * n + K,
                                ccol + n * 512: ccol + (n + 1) * 512],
                        start=True, stop=True,
                        tile_position=(32 * n, 0))
                dst = quad[:, cb * CBW:(cb + 1) * CBW]
                if cb in dve_cbs:
                    # DVE eviction: strip + its rowmax in one 1x pass
                    nc.vector.tensor_scalar(
                        out=dst, in0=psum[:], scalar1=0.0, scalar2=None,
                        op0=ADD, op1=MAX,
                        accum_out=rfin[:, rbase + cb:rbase + cb + 1])
                else:
                    nc.scalar.activation(dst, psum[:],
                                         mybir.ActivationFunctionType.Copy)
            # rowmax of the contiguous ACT-evicted span (4x fp16 pass)
            aw = (ahi - alo) * CBW
            nc.vector.tensor_scalar(
                out=dummy[:, :aw], in0=quad[:, alo * CBW:ahi * CBW],
                scalar1=0.0, scalar2=None, op0=ADD, op1=MAX,
                accum_out=rfin[:, rbase + alo:rbase + alo + 1])
            if not init:
                if t == RT - 1:
                    # final fold per 2048-slice so the colmax DMA overlaps
                    for cb in range(CB):
                        sl = slice(cb * CBW, (cb + 1) * CBW)
                        nc.vector.scalar_tensor_tensor(
                            out=running[b][:, sl], in0=quad[:, sl],
                            scalar=0.0, in1=running[b][:, sl],
                            op0=ADD, op1=MAX)
                        nc.sync.dma_start(cmax_d[:, b * N + cb * CBW:
                                                 b * N + (cb + 1) * CBW],
                                          running[b][:, sl])
                else:
                    nc.vector.scalar_tensor_tensor(
                        out=running[b][:], in0=quad[:], scalar=0.0,
                        in1=running[b][:], op0=ADD, op1=MAX)
        nc.sync.dma_start(rmax_d[:], rfin[:])

    nc.compile()
    return nc


_NC_CACHE = None


def _get_nc():
    global _NC_CACHE
    if _NC_CACHE is None:
        _NC_CACHE = build_nc()
    return _NC_CACHE


def make_in_maps(pred, gt):
    """Per-core input dicts. Core c gets gt rows [c*GPC, (c+1)*GPC) of each
    batch (aug_gt columns laid out b-major: (b*RT + t)*128 + p)."""
    aug_gt, aug_pred = _build_aug(pred, gt)
    ag_bn = aug_gt.reshape(K, B, N)
    in_maps = []
    for c in range(NCORES):
        ag_c = ag_bn[:, :, c * GPC:(c + 1) * GPC].reshape(K, B * GPC)
        in_maps.append({"aug_gt": np.ascontiguousarray(ag_c),
                        "aug_pred": np.ascontiguousarray(aug_pred)})
    return in_maps


def finalize(results):
    """Host finale: negated maxes -> mins -> sqrt -> means."""
    dist1_sq = np.empty((B, N), np.float64)
    for c in range(NCORES):
        r = np.asarray(results[c]["rowmax_out"], np.float64)  # [128, B*RT*4]
        r = r.reshape(128, B, RT, 4).max(axis=3)              # [128, B, RT]
        r = r.transpose(1, 2, 0).reshape(B, GPC)
        dist1_sq[:, c * GPC:(c + 1) * GPC] = -r
    call = np.stack([np.asarray(results[c]["colmax_out"])
                     for c in range(NCORES)], axis=0)  # [NC, 128, B*N]
    call = call.reshape(NCORES, 128, B, N)
    dist2_sq = -(call.max(axis=(0, 1)).astype(np.float64))

    dist1 = np.sqrt(np.maximum(dist1_sq, 0.0))
    dist2 = np.sqrt(np.maximum(dist2_sq, 0.0))
    chamfer = (dist1.mean(axis=1) + dist2.mean(axis=1)).mean()
    return np.float32(chamfer)


def kernel(pred, gt):
    _ensure_concourse()
    pred = np.asarray(pred, dtype=np.float32)
    gt = np.asarray(gt, dtype=np.float32)
    assert pred.shape == (B, N, 3) and gt.shape == (B, N, 3)

    in_maps = make_in_maps(pred, gt)
    nc = _get_nc()
    from concourse import bass_utils
    res = bass_utils.run_bass_kernel_spmd(nc, in_maps, core_ids=list(range(NCORES)))
    return finalize(res.results)
